# revision 15
# baseline (speedup 1.0000x reference)
"""CorrCosine TRN2 kernel (v4).

out[b, i, j, h, w] = <cur[b,:,i,j]/||cur[b,:,i,j]||, ref[b,:,h,w]/||ref[b,:,h,w]||>

Data-parallel over batch B=8 across the 8 NeuronCores; per core one
[4096 x 256] @ [256 x 4096] GEMM in bf16. L2 normalization (~0.1% of
FLOPs) is applied on the host in fp32 (same EPS semantics as the
reference) during input prep, like the host bf16 cast.

Device kernel = pure GEMM, tuned for the DMA system as much as the PE:
- 2 stripes of 2048 output cols x 32 row-tiles. Each psum tile is
  [128, 2048] fp32 = 4 banks; bufs=2 uses all 8 banks. 2048-wide
  output rows are 4KB-contiguous in HBM, so every DMA descriptor is
  4KB instead of 2KB -- this lifts the per-core DMA ceiling from
  ~300 GB/s to ~360 GB/s, which the output stream needs (33.5MB over
  ~118us of GEMM = 284 GB/s + 4.2MB input).
- PSUM evacuation fp32->bf16 alternates ACT (Copy) / DVE (tensor_copy):
  2.2-2.4us per 2048-tile vs a 3.46us per-engine cadence.
- 64 output DMAs of 512KB alternate sync/gpsimd.
- inputs at 1024-wide: ref b0,b1 + cur b0..b3 up front (~10.6-15us),
  ref b2,b3 lazily during stripe 0 (only needed by stripe 1, ~70us).
"""

import numpy as np
import ml_dtypes

from concourse import bacc, mybir
from concourse import tile
from concourse.bass_utils import run_bass_kernel_spmd

B, C, H, W = 8, 256, 64, 64
HW = H * W            # 4096
P = 128               # partitions
KT = C // P           # 2 k-tiles
FD = 512              # psum bank free dim (fp32)
SW = 2048             # stripe width
NS = HW // SW         # 2 stripes
MT = HW // P          # 32 m-tiles
BW = 1024             # input DMA block width

f32 = mybir.dt.float32
bf16 = mybir.dt.bfloat16
AF = mybir.ActivationFunctionType

_cached_nc = None


def _build():
    nc = bacc.Bacc("TRN2", target_bir_lowering=False, debug=False)
    cur_d = nc.dram_tensor("cur", [C, HW], bf16, kind="ExternalInput")
    ref_d = nc.dram_tensor("ref", [C, HW], bf16, kind="ExternalInput")
    out_d = nc.dram_tensor("out", [HW, HW], bf16, kind="ExternalOutput")

    with tile.TileContext(nc) as tc:
        with (
            tc.tile_pool(name="dat", bufs=1) as datp,
            tc.tile_pool(name="ps", bufs=8, space="PSUM") as psp,
            tc.tile_pool(name="outp", bufs=14) as obp,
        ):
            raw = {}
            for t in ("ref", "cur"):
                for k in range(KT):
                    raw[t, k] = datp.tile(
                        [P, HW], bf16, tag=f"raw_{t}{k}", name=f"raw_{t}{k}"
                    )

            def in_dma(t, sl, q):
                src_d = ref_d if t == "ref" else cur_d
                for k in range(KT):
                    q.dma_start(raw[t, k][:, sl], src_d[k * P:(k + 1) * P, sl])

            # early inputs on the sync ring, criticality order, 512 first.
            # stripe 0 reads ref cols 0:2048 = blocks b0+b1; ref b2/b3 are
            # needed only from stripe 1 (~70us) and arrive lazily on gpsimd.
            in_dma("ref", slice(0, FD), nc.sync)
            in_dma("cur", slice(0, FD), nc.sync)
            in_dma("ref", slice(FD, BW), nc.sync)
            in_dma("cur", slice(FD, BW), nc.sync)
            in_dma("ref", slice(BW, 2 * BW), nc.sync)
            for b in range(1, 4):
                in_dma("cur", slice(b * BW, (b + 1) * BW), nc.sync)

            ei = 0
            for s in range(NS):
                for m in range(MT):
                    if s == 0 and m in (8, 14):
                        b = m // 4
                        in_dma("ref", slice(b * BW, (b + 1) * BW), nc.gpsimd)
                    msl = slice(m * P, (m + 1) * P)
                    ssl = slice(s * SW, (s + 1) * SW)
                    ob = obp.tile([P, SW], bf16, tag="ob", name="ob")
                    for h in range(2):   # half-tiles: A = c0/c1, B = c2/c3
                        hsl = slice(h * SW // 2, (h + 1) * SW // 2)
                        pt = psp.tile([P, SW // 2], f32, tag=f"pt{h}",
                                      name="pt", bufs=2)
                        # k-outer: one stationary load serves both chunks
                        for k in range(KT):
                            for cc in range(2):
                                c = 2 * h + cc
                                nsl = slice(s * SW + c * FD,
                                            s * SW + (c + 1) * FD)
                                nc.tensor.matmul(
                                    pt[:, cc * FD:(cc + 1) * FD],
                                    raw["cur", k][:, msl],
                                    raw["ref", k][:, nsl],
                                    start=(k == 0), stop=(k == KT - 1),
                                )
                        if (ei + h) % 2 == 0:
                            nc.scalar.activation(ob[:, hsl], pt[:, :], AF.Copy)
                        else:
                            nc.vector.tensor_copy(ob[:, hsl], pt[:, :])
                    # queue balance: sync also carries 3.2MB of inputs, so
                    # gpsimd takes two extra output tiles (16 and 48)
                    on_sync = (ei % 2 == 0) and ei not in (16, 48)
                    queue = nc.sync if on_sync else nc.gpsimd
                    queue.dma_start(out_d[msl, ssl], ob[:, :])
                    ei += 1

    nc.compile()
    return nc


def _get_nc():
    global _cached_nc
    if _cached_nc is None:
        _cached_nc = _build()
    return _cached_nc


def _run(cur, ref, trace=False, **kw):
    """cur/ref: [B, C, HW] bf16 (pre-normalized). Returns (out f32, res)."""
    nc = _get_nc()
    cur = np.asarray(cur).astype(ml_dtypes.bfloat16)
    ref = np.asarray(ref).astype(ml_dtypes.bfloat16)
    in_maps = [{"cur": cur[b], "ref": ref[b]} for b in range(B)]
    res = run_bass_kernel_spmd(nc, in_maps, list(range(B)), trace=trace, **kw)
    out = np.stack(
        [res.results[b]["out"].astype(np.float32) for b in range(B)]
    )
    return out, res


def _l2n(x):
    """L2-normalize along axis 1 with the reference EPS semantics."""
    n = np.sqrt((x * x).sum(axis=1, keepdims=True))
    return x / np.maximum(n, 1e-12)


def kernel(ref_features, cur_features):
    ref = np.asarray(ref_features, np.float32).reshape(B, C, HW)
    cur = np.asarray(cur_features, np.float32).reshape(B, C, HW)
    out, _ = _run(_l2n(cur), _l2n(ref))
    return out.reshape(B, H, W, H, W)


# revision 20
# speedup vs baseline: 1.0136x; 1.0136x over previous
"""CorrCosine TRN2 kernel (v4).

out[b, i, j, h, w] = <cur[b,:,i,j]/||cur[b,:,i,j]||, ref[b,:,h,w]/||ref[b,:,h,w]||>

Data-parallel over batch B=8 across the 8 NeuronCores; per core one
[4096 x 256] @ [256 x 4096] GEMM in bf16. L2 normalization (~0.1% of
FLOPs) is applied on the host in fp32 (same EPS semantics as the
reference) during input prep, like the host bf16 cast.

Device kernel = pure GEMM, tuned for the DMA system as much as the PE:
- 2 stripes of 2048 output cols x 32 row-tiles. Each psum tile is
  [128, 2048] fp32 = 4 banks; bufs=2 uses all 8 banks. 2048-wide
  output rows are 4KB-contiguous in HBM, so every DMA descriptor is
  4KB instead of 2KB -- this lifts the per-core DMA ceiling from
  ~300 GB/s to ~360 GB/s, which the output stream needs (33.5MB over
  ~118us of GEMM = 284 GB/s + 4.2MB input).
- PSUM evacuation fp32->bf16 alternates ACT (Copy) / DVE (tensor_copy):
  2.2-2.4us per 2048-tile vs a 3.46us per-engine cadence.
- 64 output DMAs of 512KB alternate sync/gpsimd.
- inputs at 1024-wide: ref b0,b1 + cur b0..b3 up front (~10.6-15us),
  ref b2,b3 lazily during stripe 0 (only needed by stripe 1, ~70us).
"""

import numpy as np
import ml_dtypes

from concourse import bacc, mybir
from concourse import tile
from concourse.bass_utils import run_bass_kernel_spmd

B, C, H, W = 8, 256, 64, 64
HW = H * W            # 4096
P = 128               # partitions
KT = C // P           # 2 k-tiles
FD = 512              # psum bank free dim (fp32)
SW = 2048             # stripe width
NS = HW // SW         # 2 stripes
MT = HW // P          # 32 m-tiles
BW = 1024             # input DMA block width

f32 = mybir.dt.float32
bf16 = mybir.dt.bfloat16
AF = mybir.ActivationFunctionType

_cached_nc = None


def _build():
    nc = bacc.Bacc("TRN2", target_bir_lowering=False, debug=False)
    cur_d = nc.dram_tensor("cur", [C, HW], bf16, kind="ExternalInput")
    ref_d = nc.dram_tensor("ref", [C, HW], bf16, kind="ExternalInput")
    out_d = nc.dram_tensor("out", [HW, HW], bf16, kind="ExternalOutput")

    with tile.TileContext(nc) as tc:
        with (
            tc.tile_pool(name="dat", bufs=1) as datp,
            tc.tile_pool(name="ps", bufs=8, space="PSUM") as psp,
            tc.tile_pool(name="outp", bufs=8) as obp,
        ):
            raw = {}
            for t in ("ref", "cur"):
                for k in range(KT):
                    raw[t, k] = datp.tile(
                        [P, HW], bf16, tag=f"raw_{t}{k}", name=f"raw_{t}{k}"
                    )

            def in_dma(t, sl, q):
                src_d = ref_d if t == "ref" else cur_d
                for k in range(KT):
                    q.dma_start(raw[t, k][:, sl], src_d[k * P:(k + 1) * P, sl])

            # early inputs on the sync ring, criticality order; the very
            # first slices are 256 wide so the PE can start ~10us in.
            # stripe 0 reads ref cols 0:2048 = blocks b0+b1; ref b2/b3 are
            # needed only from stripe 1 (~70us) and arrive lazily on gpsimd.
            in_dma("ref", slice(0, 256), nc.sync)
            in_dma("cur", slice(0, 256), nc.sync)
            in_dma("ref", slice(256, FD), nc.sync)
            in_dma("cur", slice(256, FD), nc.sync)
            in_dma("ref", slice(FD, BW), nc.sync)
            in_dma("cur", slice(FD, BW), nc.sync)
            in_dma("ref", slice(BW, 2 * BW), nc.sync)
            for b in range(1, 4):
                in_dma("cur", slice(b * BW, (b + 1) * BW), nc.sync)

            ei = 0
            for s in range(NS):
                for m in range(MT):
                    if s == 0 and m in (8, 14):
                        b = m // 4
                        in_dma("ref", slice(b * BW, (b + 1) * BW), nc.gpsimd)
                    msl = slice(m * P, (m + 1) * P)
                    ssl = slice(s * SW, (s + 1) * SW)
                    ob = obp.tile([P, SW], bf16, tag="ob", name="ob")
                    for h in range(2):   # half-tiles: A = c0/c1, B = c2/c3
                        hsl = slice(h * SW // 2, (h + 1) * SW // 2)
                        pt = psp.tile([P, SW // 2], f32, tag=f"pt{h}",
                                      name="pt", bufs=2)
                        # k-outer: one stationary load serves both chunks
                        for k in range(KT):
                            for cc in range(2):
                                c = 2 * h + cc
                                nsl = slice(s * SW + c * FD,
                                            s * SW + (c + 1) * FD)
                                nc.tensor.matmul(
                                    pt[:, cc * FD:(cc + 1) * FD],
                                    raw["cur", k][:, msl],
                                    raw["ref", k][:, nsl],
                                    start=(k == 0), stop=(k == KT - 1),
                                )
                        if (ei + h) % 2 == 0:
                            nc.scalar.activation(ob[:, hsl], pt[:, :], AF.Copy)
                        else:
                            nc.vector.tensor_copy(ob[:, hsl], pt[:, :])
                    # queue balance: sync also carries 3.2MB of inputs, so
                    # gpsimd takes two extra output tiles (16 and 48)
                    on_sync = (ei % 2 == 0) and ei not in (16, 48)
                    queue = nc.sync if on_sync else nc.gpsimd
                    if ei >= 62:
                        # last tiles: split across both queues for a
                        # parallel drain
                        nc.sync.dma_start(out_d[msl, s * SW:s * SW + SW // 2],
                                          ob[:, 0:SW // 2])
                        nc.gpsimd.dma_start(out_d[msl, s * SW + SW // 2:(s + 1) * SW],
                                            ob[:, SW // 2:SW])
                    else:
                        queue.dma_start(out_d[msl, ssl], ob[:, :])
                    ei += 1

    nc.compile()
    return nc


def _get_nc():
    global _cached_nc
    if _cached_nc is None:
        _cached_nc = _build()
    return _cached_nc


def _run(cur, ref, trace=False, **kw):
    """cur/ref: [B, C, HW] bf16 (pre-normalized). Returns (out f32, res)."""
    nc = _get_nc()
    cur = np.asarray(cur).astype(ml_dtypes.bfloat16)
    ref = np.asarray(ref).astype(ml_dtypes.bfloat16)
    in_maps = [{"cur": cur[b], "ref": ref[b]} for b in range(B)]
    res = run_bass_kernel_spmd(nc, in_maps, list(range(B)), trace=trace, **kw)
    out = np.stack(
        [res.results[b]["out"].astype(np.float32) for b in range(B)]
    )
    return out, res


def _l2n(x):
    """L2-normalize along axis 1 with the reference EPS semantics."""
    n = np.sqrt((x * x).sum(axis=1, keepdims=True))
    return x / np.maximum(n, 1e-12)


def kernel(ref_features, cur_features):
    ref = np.asarray(ref_features, np.float32).reshape(B, C, HW)
    cur = np.asarray(cur_features, np.float32).reshape(B, C, HW)
    out, _ = _run(_l2n(cur), _l2n(ref))
    return out.reshape(B, H, W, H, W)


# revision 21
# speedup vs baseline: 1.0194x; 1.0057x over previous
"""CorrCosine TRN2 kernel (v4).

out[b, i, j, h, w] = <cur[b,:,i,j]/||cur[b,:,i,j]||, ref[b,:,h,w]/||ref[b,:,h,w]||>

Data-parallel over batch B=8 across the 8 NeuronCores; per core one
[4096 x 256] @ [256 x 4096] GEMM in bf16. L2 normalization (~0.1% of
FLOPs) is applied on the host in fp32 (same EPS semantics as the
reference) during input prep, like the host bf16 cast.

Device kernel = pure GEMM, tuned for the DMA system as much as the PE:
- 2 stripes of 2048 output cols x 32 row-tiles. Each psum tile is
  [128, 2048] fp32 = 4 banks; bufs=2 uses all 8 banks. 2048-wide
  output rows are 4KB-contiguous in HBM, so every DMA descriptor is
  4KB instead of 2KB -- this lifts the per-core DMA ceiling from
  ~300 GB/s to ~360 GB/s, which the output stream needs (33.5MB over
  ~118us of GEMM = 284 GB/s + 4.2MB input).
- PSUM evacuation fp32->bf16 alternates ACT (Copy) / DVE (tensor_copy):
  2.2-2.4us per 2048-tile vs a 3.46us per-engine cadence.
- 64 output DMAs of 512KB alternate sync/gpsimd.
- inputs at 1024-wide: ref b0,b1 + cur b0..b3 up front (~10.6-15us),
  ref b2,b3 lazily during stripe 0 (only needed by stripe 1, ~70us).
"""

import numpy as np
import ml_dtypes

from concourse import bacc, mybir
from concourse import tile
from concourse.bass_utils import run_bass_kernel_spmd

B, C, H, W = 8, 256, 64, 64
HW = H * W            # 4096
P = 128               # partitions
KT = C // P           # 2 k-tiles
FD = 512              # psum bank free dim (fp32)
SW = 2048             # stripe width
NS = HW // SW         # 2 stripes
MT = HW // P          # 32 m-tiles
BW = 1024             # input DMA block width

f32 = mybir.dt.float32
bf16 = mybir.dt.bfloat16
AF = mybir.ActivationFunctionType

_cached_nc = None


def _build():
    nc = bacc.Bacc("TRN2", target_bir_lowering=False, debug=False)
    cur_d = nc.dram_tensor("cur", [C, HW], bf16, kind="ExternalInput")
    ref_d = nc.dram_tensor("ref", [C, HW], bf16, kind="ExternalInput")
    out_d = nc.dram_tensor("out", [HW, HW], bf16, kind="ExternalOutput")

    with tile.TileContext(nc) as tc:
        with (
            tc.tile_pool(name="dat", bufs=1) as datp,
            tc.tile_pool(name="ps", bufs=8, space="PSUM") as psp,
            tc.tile_pool(name="outp", bufs=8) as obp,
        ):
            raw = {}
            for t in ("ref", "cur"):
                for k in range(KT):
                    raw[t, k] = datp.tile(
                        [P, HW], bf16, tag=f"raw_{t}{k}", name=f"raw_{t}{k}"
                    )

            def in_dma(t, sl, q):
                src_d = ref_d if t == "ref" else cur_d
                for k in range(KT):
                    q.dma_start(raw[t, k][:, sl], src_d[k * P:(k + 1) * P, sl])

            # early inputs on the sync ring, criticality order; the very
            # first slices are 256 wide so the PE can start ~10us in.
            # stripe 0 reads ref cols 0:2048 = blocks b0+b1; ref b2/b3 are
            # needed only from stripe 1 (~70us) and arrive lazily on gpsimd.
            in_dma("ref", slice(0, FD), nc.sync)
            in_dma("cur", slice(0, FD), nc.sync)
            in_dma("ref", slice(FD, BW), nc.sync)
            in_dma("cur", slice(FD, BW), nc.sync)
            in_dma("ref", slice(BW, 2 * BW), nc.sync)
            for b in range(1, 4):
                in_dma("cur", slice(b * BW, (b + 1) * BW), nc.sync)

            ei = 0
            for s in range(NS):
                for m in range(MT):
                    if s == 0 and m in (8, 14):
                        b = m // 4
                        in_dma("ref", slice(b * BW, (b + 1) * BW), nc.gpsimd)
                    msl = slice(m * P, (m + 1) * P)
                    ssl = slice(s * SW, (s + 1) * SW)
                    ob = obp.tile([P, SW], bf16, tag="ob", name="ob")
                    for h in range(2):   # half-tiles: A = c0/c1, B = c2/c3
                        hsl = slice(h * SW // 2, (h + 1) * SW // 2)
                        pt = psp.tile([P, SW // 2], f32, tag=f"pt{h}",
                                      name="pt", bufs=2)
                        # k-outer: one stationary load serves both chunks
                        for k in range(KT):
                            for cc in range(2):
                                c = 2 * h + cc
                                nsl = slice(s * SW + c * FD,
                                            s * SW + (c + 1) * FD)
                                nc.tensor.matmul(
                                    pt[:, cc * FD:(cc + 1) * FD],
                                    raw["cur", k][:, msl],
                                    raw["ref", k][:, nsl],
                                    start=(k == 0), stop=(k == KT - 1),
                                )
                        if (ei + h) % 2 == 0:
                            nc.scalar.activation(ob[:, hsl], pt[:, :], AF.Copy)
                        else:
                            nc.vector.tensor_copy(ob[:, hsl], pt[:, :])
                    # queue balance: sync also carries 3.2MB of inputs, so
                    # gpsimd takes two extra output tiles (16 and 48)
                    on_sync = (ei % 2 == 0) and ei not in (16, 48)
                    queue = nc.sync if on_sync else nc.gpsimd
                    if ei >= 62:
                        # last tiles: split across both queues for a
                        # parallel drain
                        nc.sync.dma_start(out_d[msl, s * SW:s * SW + SW // 2],
                                          ob[:, 0:SW // 2])
                        nc.gpsimd.dma_start(out_d[msl, s * SW + SW // 2:(s + 1) * SW],
                                            ob[:, SW // 2:SW])
                    else:
                        queue.dma_start(out_d[msl, ssl], ob[:, :])
                    ei += 1

    nc.compile()
    return nc


def _get_nc():
    global _cached_nc
    if _cached_nc is None:
        _cached_nc = _build()
    return _cached_nc


def _run(cur, ref, trace=False, **kw):
    """cur/ref: [B, C, HW] bf16 (pre-normalized). Returns (out f32, res)."""
    nc = _get_nc()
    cur = np.asarray(cur).astype(ml_dtypes.bfloat16)
    ref = np.asarray(ref).astype(ml_dtypes.bfloat16)
    in_maps = [{"cur": cur[b], "ref": ref[b]} for b in range(B)]
    res = run_bass_kernel_spmd(nc, in_maps, list(range(B)), trace=trace, **kw)
    out = np.stack(
        [res.results[b]["out"].astype(np.float32) for b in range(B)]
    )
    return out, res


def _l2n(x):
    """L2-normalize along axis 1 with the reference EPS semantics."""
    n = np.sqrt((x * x).sum(axis=1, keepdims=True))
    return x / np.maximum(n, 1e-12)


def kernel(ref_features, cur_features):
    ref = np.asarray(ref_features, np.float32).reshape(B, C, HW)
    cur = np.asarray(cur_features, np.float32).reshape(B, C, HW)
    out, _ = _run(_l2n(cur), _l2n(ref))
    return out.reshape(B, H, W, H, W)
